# revision 1
# baseline (speedup 1.0000x reference)
"""Trainium2 Bass kernel for GBMS mean-shift step (nn_GBMS_RNN_137438953906).

Math (per batch b):
    W = exp((X X^T - 1) / bandwidth^2)          [N, N]
    Y = (W @ X) / rowsum(W)                     [N, D]
    out = Y / max(||Y||_2, 1e-12)  (L2 norm along D)

Key identity: rowsum(W) is a positive per-row scalar, so it cancels in the
final L2 normalization:  l2norm(W@X / d) == l2norm(W@X).  We therefore never
compute row sums.

Sharding: data-parallel over batch B=8 across the 8 NeuronCores (1 batch
each).  Within a core, flash-attention-style row blocking: W is produced in
[128, 512] PSUM tiles and consumed immediately; the full NxN matrix is never
materialized.

Per-core dataflow:
  XT[d, n] = X^T / |bandwidth|      (PE transposes; scale folded in so the
                                     S matmul directly yields X X^T / b^2)
  for each group g of 512 output rows:
      for jb in 32 blocks of 128:
          S[j, n512] = XT[:,jb128]^T @ XT[:,g512]          (fp32r matmul)
          W[j, n512] = exp(S - 1/b^2)                      (ACT, 3-bank batches)
          YT[d, n512] += Xnat[:,jb,:]^T @ W                (fp32r matmul, PSUM acc)
      transpose YT -> Y[n, d] tiles, accumulate sum-of-squares
  1/norm = fast-inverse-sqrt(ss) + 2 Newton steps (DVE-only, no ACT tables)
  out = Y * (1/norm)

Symmetry reuse: W is symmetric and fp32r S matmuls are exactly symmetric
(identical rounded operands), so each group saves the 4 W tiles whose j-range
equals the NEXT group's columns; the next group reconstructs its 4 matching
tiles by PE-transposing them, skipping their S matmul and exp (-12.5% of the
exp work on 7 of 8 groups, relieving the bottleneck ACT engine).
"""

import sys

if "/opt/trn_rl_repo" not in sys.path:
    sys.path.insert(0, "/opt/trn_rl_repo")

import numpy as np

import concourse.mybir as mybir
from concourse import bacc
from concourse.tile import TileContext
from concourse.bass_utils import run_bass_kernel_spmd
from concourse.masks import make_identity

P = 128
N = 4096
D = 128
NB = N // P  # 32 row blocks
G = N // 512  # 8 column groups
JT = 3  # j-blocks per exp batch (3 PSUM banks)
NCHUNK = 8  # input DMA chunks (4 row-blocks each)

F32 = mybir.dt.float32
F32R = mybir.dt.float32r

S_MM_DT = F32R
Y_MM_DT = F32R

_CACHED_NC = None


def _build():
    nc = bacc.Bacc("TRN2", target_bir_lowering=False, debug=False)

    x_in = nc.dram_tensor("X", [N, D], F32, kind="ExternalInput")
    bw_in = nc.dram_tensor("bandwidth", [1], F32, kind="ExternalInput")
    y_out = nc.dram_tensor("Y", [N, D], F32, kind="ExternalOutput")

    x_src = x_in.rearrange("(jb p) d -> p jb d", p=P)  # [128, 32, 128] view
    y_dst = y_out.rearrange("(nb p) d -> p nb d", p=P)

    with TileContext(nc) as tc:
        with (
            tc.tile_pool(name="const", bufs=1) as const,
            tc.tile_pool(name="spsum", bufs=2, space="PSUM") as s_pool,
            tc.tile_pool(name="ytpsum", bufs=1, space="PSUM") as yt_pool,
            tc.tile_pool(name="tppsum", bufs=1, space="PSUM") as tp_pool,
            tc.tile_pool(name="wpool", bufs=5) as w_pool,
            tc.tile_pool(name="svpool", bufs=2) as sv_pool,
            tc.tile_pool(name="stgpool", bufs=2) as stg_pool,
            tc.tile_pool(name="sqpool", bufs=2) as sq_pool,
        ):
            # ---- input chunk 0 first: it gates the whole pipeline and DMA
            # dispatches serialize on the SP sequencer (~500ns each) ----
            x_nat = const.tile([P, NB, D], F32)  # [j_in_block, jb, d]
            cb = NB // NCHUNK  # row blocks per chunk
            # chunk 0 takes the fast HWDGE path, dispatched first; the tiny
            # bandwidth scalar rides the GPSIMD queue (SWDGE trickle rate is
            # irrelevant for 4 bytes but would starve chunk-0 transposes)
            nc.sync.dma_start(x_nat[:, 0:cb, :], x_src[:, 0:cb, :])

            bw = const.tile([P, 1], F32)
            nc.gpsimd.dma_start(bw[:], bw_in[None, :].to_broadcast([P, 1]))

            # identity built on the otherwise-idle GPSIMD engine (no DMA slot)
            ident = const.tile([P, P], F32)
            make_identity(nc, ident[:])
            # fp32r view for transposing fp32r W tiles (symmetry reuse)
            ident_r = const.tile([P, P], F32R)
            nc.vector.tensor_copy(ident_r[:], ident[:])

            for c in range(1, NCHUNK):
                nc.sync.dma_start(
                    x_nat[:, c * cb : (c + 1) * cb, :],
                    x_src[:, c * cb : (c + 1) * cb, :],
                )

            scr = const.tile([P, 5], F32)
            negb = scr[:, 0:1]
            absb = scr[:, 1:2]
            rb = scr[:, 2:3]
            negc = scr[:, 3:4]
            dummy = scr[:, 4:5]
            nc.vector.tensor_scalar_mul(negb, bw[:], -1.0)
            nc.vector.tensor_tensor(absb, bw[:], negb, mybir.AluOpType.max)
            nc.vector.reciprocal(rb, absb)  # 1/|b|
            nc.vector.tensor_tensor(negc, rb, rb, mybir.AluOpType.mult)
            nc.vector.tensor_scalar_mul(negc, negc, -1.0)  # -1/b^2

            # Preload the exp ACT table set while DMAs stream in -- the only
            # table load in the kernel (normalization is DVE-only).
            nc.scalar.activation(dummy, absb, mybir.ActivationFunctionType.Exp)

            # Junk transposes to start ramping the PE clock (HAM) while the
            # first input chunk is still in flight.
            warm_ps = s_pool.tile([P, JT, 512], F32, tag="s")
            for t in range(6):
                nc.tensor.transpose(
                    warm_ps[:, t // 4, (t % 4) * P : (t % 4 + 1) * P],
                    ident[:],
                    ident[:],
                )

            # fp32r copy of x_nat for the Y matmul (fp32r matmul operands
            # must be written pre-rounded by their producer)
            x_natr = const.tile([P, NB, D], Y_MM_DT)
            # XT = X^T / |b|, built per chunk via PE transposes
            xt = const.tile([P, N], S_MM_DT)

            chunks_done = [0]

            def emit_chunk(c, per_block=False):
                # 4 PE transposes -> one PSUM bank -> scaled copy to xt.
                # per_block pipelines transpose/copy at row-block granularity
                # (used for chunk 0, which gates the very first exp).
                if not per_block:
                    nc.vector.tensor_copy(
                        x_natr[:, c * cb : (c + 1) * cb, :],
                        x_nat[:, c * cb : (c + 1) * cb, :],
                    )
                # chunks after the first transpose the pre-rounded fp32r copy
                # (1.5 cyc/row vs 2.0); chunk 0 keeps the fp32 path so the
                # x_natr copy stays off the startup critical path
                tp_dt = F32 if per_block else F32R
                t_ps = s_pool.tile([P, JT, 512], tp_dt, tag="s")
                for o in range(cb):
                    jb = c * cb + o
                    if per_block:
                        # alternate PSUM banks so transpose o+1 doesn't hit a
                        # bank-level WAR against the copy reading bank o
                        dst = t_ps[:, o % JT, 0:P]
                        nc.tensor.transpose(dst, x_nat[:, jb, :], ident[:])
                    else:
                        dst = t_ps[:, 0, o * P : (o + 1) * P]
                        nc.tensor.transpose(dst, x_natr[:, jb, :], ident_r[:])
                    if per_block:
                        nc.vector.tensor_scalar_mul(
                            xt[:, jb * P : (jb + 1) * P], dst, rb
                        )
                if not per_block:
                    nc.vector.tensor_scalar_mul(
                        xt[:, c * cb * P : (c + 1) * cb * P], t_ps[:, 0, :], rb
                    )
                else:
                    # x_natr is only needed by the (later) Y matmuls -- keep
                    # it off the startup critical path
                    nc.vector.tensor_copy(
                        x_natr[:, c * cb : (c + 1) * cb, :],
                        x_nat[:, c * cb : (c + 1) * cb, :],
                    )

            def need_chunks(upto):
                while chunks_done[0] <= min(upto, NCHUNK - 1):
                    emit_chunk(chunks_done[0], per_block=(chunks_done[0] == 0))
                    chunks_done[0] += 1

            # ---- output staging ----
            y_all = const.tile([P, NB, D], F32)  # [n_in_block, nb, d]
            ss_all = const.tile([P, NB], F32)
            half = const.tile([P, NB], F32)
            tmp = const.tile([P, NB], F32)
            rcp = const.tile([P, NB], F32)
            I32 = mybir.dt.int32
            magic = const.tile([P, NB], I32)
            shreg = const.tile([P, NB], I32)
            nc.vector.memset(magic[:], 0x5F3759DF)

            def normalize_and_store(g0, g1):
                """L2-normalize output rows of groups [g0, g1) and DMA out.

                1/norm = rsqrt(ss) via the fast-inverse-sqrt bit trick plus
                2 Newton iterations -- DVE-only, no ACT table switches, and
                ~4e-6 relative accuracy.  ss == 0 rows stay finite (y == 0
                there, matching the reference's eps-guarded division).
                """
                lo, hi = g0 * 4, g1 * 4  # nb range
                ss = ss_all[:, lo:hi]
                rs = rcp[:, lo:hi]
                hf = half[:, lo:hi]
                tm = tmp[:, lo:hi]
                nc.vector.tensor_scalar(
                    shreg[:, lo:hi],
                    ss.bitcast(I32),
                    1,
                    None,
                    mybir.AluOpType.logical_shift_right,
                )
                nc.vector.tensor_tensor(
                    rs.bitcast(I32),
                    magic[:, lo:hi],
                    shreg[:, lo:hi],
                    mybir.AluOpType.subtract,
                )
                nc.vector.tensor_scalar_mul(hf, ss, 0.5)
                for _ in range(2):
                    nc.vector.tensor_tensor(tm, rs, rs, mybir.AluOpType.mult)
                    nc.vector.tensor_tensor(tm, tm, hf, mybir.AluOpType.mult)
                    nc.vector.tensor_scalar(
                        tm, tm, -1.0, 1.5, mybir.AluOpType.mult, mybir.AluOpType.add
                    )
                    nc.vector.tensor_tensor(rs, rs, tm, mybir.AluOpType.mult)
                for nb in range(lo, hi):
                    nc.vector.tensor_scalar_mul(
                        y_all[:, nb, :], y_all[:, nb, :], rcp[:, nb : nb + 1]
                    )
                # split the store across DMA queues
                mid = (lo + hi) // 2
                nc.sync.dma_start(y_dst[:, lo:mid, :], y_all[:, lo:mid, :])
                nc.sync.dma_start(y_dst[:, mid:hi, :], y_all[:, mid:hi, :])

            def make_tail(g, yt, fine=False):
                """Tail of group g: YT[d, n512] -> Y[n, d] + sum of squares.
                Emitted 2 batches into the NEXT group so the 4 PE transposes
                hide behind that group's ACT work instead of stalling it.
                fine=True half-pipelines the chain (used for the last group,
                where the tail is the end-to-end critical path)."""

                def tail():
                    halves = 2 if fine else 1
                    hw = 4 // halves  # row blocks per half
                    stg = stg_pool.tile([P, 512], F32, tag="stg")
                    tp = tp_pool.tile([P, 4, P], F32, tag="tp")
                    sq = sq_pool.tile([P, 4, P], F32, tag="sq")
                    for h in range(halves):
                        blk = slice(h * hw, (h + 1) * hw)
                        nc.vector.tensor_copy(
                            stg[:, h * hw * P : (h + 1) * hw * P],
                            yt[:, h * hw * P : (h + 1) * hw * P],
                        )
                        for t in range(h * hw, (h + 1) * hw):
                            nc.tensor.transpose(
                                tp[:, t, :], stg[:, t * P : (t + 1) * P], ident[:]
                            )
                        y_slice = y_all[:, g * 4 + h * hw : g * 4 + (h + 1) * hw, :]
                        nc.vector.tensor_copy(y_slice, tp[:, blk, :])
                        nc.vector.tensor_tensor(
                            sq[:, blk, :], y_slice, y_slice, mybir.AluOpType.mult
                        )
                        nc.vector.tensor_reduce(
                            ss_all[:, g * 4 + h * hw : g * 4 + (h + 1) * hw],
                            sq[:, blk, :],
                            axis=mybir.AxisListType.X,
                            op=mybir.AluOpType.add,
                        )

                return tail

            def emit_y(py):
                jbs, rhss, flags, pyt = py
                for jb_, rhs_, (fst, lst) in zip(jbs, rhss, flags):
                    nc.tensor.matmul(
                        pyt[:], x_natr[:, jb_, :], rhs_, start=fst, stop=lst
                    )

            def build_walk(g):
                """Per-group j-walk as (kind, jbs) batches.

                W is symmetric and fp32r S matmuls are exactly symmetric
                (identical rounded operands, same accumulation order), so the
                4 tiles whose j-range equals the NEXT group's columns are
                saved ("save") and the next group reconstructs its matching 4
                tiles by PE-transposing them ("reuse") -- skipping their S
                matmul and exp entirely.  Reuse batches are interleaved so
                ACT's queue never runs dry.
                """
                if g == 0:
                    # ascending for the input-chunk pipeline; blocks 4..7
                    # (= group 1's columns) are the save range
                    batches = [
                        ("normal", [0]),
                        ("normal", [1, 2]),
                        ("normal", [3]),
                        ("save", [4, 5]),
                        ("save", [6, 7]),
                    ]
                    rest = list(range(8, NB))
                    for i in range(0, len(rest), JT):
                        batches.append(("normal", rest[i : i + JT]))
                    return batches
                reuse = [4 * (g - 1) + i for i in range(4)]
                if g < G - 1:
                    save = [4 * (g + 1) + i for i in range(4)]
                    rest = [j for j in range(NB) if j not in reuse and j not in save]
                    r3 = [rest[i : i + 3] for i in range(0, 24, 3)]
                    return [
                        ("normal", r3[0]),
                        ("normal", r3[1]),
                        ("save", save[0:2]),
                        ("normal", r3[2]),
                        ("reuse", reuse[0:1]),
                        ("normal", r3[3]),
                        ("reuse", reuse[1:2]),
                        ("save", save[2:4]),
                        ("normal", r3[4]),
                        ("reuse", reuse[2:3]),
                        ("normal", r3[5]),
                        ("reuse", reuse[3:4]),
                        ("normal", r3[6]),
                        ("normal", r3[7]),
                    ]
                # Last group: the final batch is a reuse batch -- it needs no
                # exp, so the end-of-kernel chain (last Y matmuls -> tail)
                # stops waiting on ACT.  The other three stay spread out so
                # the 1-bank tp pool never serializes back-to-back.
                rest = [j for j in range(NB) if j not in reuse]
                r3 = [rest[i : i + 3] for i in range(0, 27, 3)] + [[rest[27]]]
                return [
                    ("normal", r3[0]),
                    ("normal", r3[1]),
                    ("normal", r3[2]),
                    ("reuse", reuse[0:1]),
                    ("normal", r3[3]),
                    ("reuse", reuse[1:2]),
                    ("normal", r3[4]),
                    ("reuse", reuse[2:3]),
                    ("normal", r3[5]),
                    ("normal", r3[6]),
                    ("normal", r3[7]),
                    ("normal", r3[8]),
                    ("normal", r3[9]),
                    ("reuse", reuse[3:4]),
                ]

            # ---- main flash loop ----
            pending_tail = None
            pending_y = []
            prev_sv = None
            for g in range(G):
                yt = yt_pool.tile([P, 512], F32, tag="yt")
                n_lo = g * 512
                if g == 0:
                    need_chunks(0)  # rhs columns for group 0

                walk = build_walk(g)
                assert sorted(sum((jbs for _, jbs in walk), [])) == list(range(NB))
                sv = None
                if g < G - 1:
                    sv = sv_pool.tile([P, 4, 512], Y_MM_DT, tag="sv", name="sv")
                sv_off = 0
                walked = 0
                jt_idx = 0
                for kind, jbs in walk:
                    tsz = len(jbs)
                    if g == 0:
                        ahead = 1 if jt_idx > 0 else 0
                        need_chunks(max(jbs) // cb + ahead)
                    flags = [
                        (walked + q == 0, walked + q == NB - 1)
                        for q in range(tsz)
                    ]
                    if kind == "reuse":
                        # reconstruct tile (jb=4(g-1)+a, cols g) from the
                        # previous group's saved tiles: block (a, b) is the
                        # transpose of saved tile b's block a.  Routed through
                        # the 1-bank tp pool (single-tile batches) so the S
                        # pipeline's PSUM slots are never held up.
                        assert tsz == 1
                        a = jbs[0] - 4 * (g - 1)
                        t_t = tp_pool.tile([P, 4, P], F32R, tag="tp")
                        for b in range(4):
                            nc.tensor.transpose(
                                t_t[:, b, :],
                                prev_sv[:, b, a * P : (a + 1) * P],
                                ident_r[:],
                            )
                        w_t = w_pool.tile([P, JT, 512], Y_MM_DT, tag="w")
                        nc.vector.tensor_copy(
                            w_t[:, 0, :],
                            t_t[:].rearrange("p a b -> p (a b)"),
                        )
                        rhss = [w_t[:, 0, :]]
                    else:
                        s_t = s_pool.tile([P, JT, 512], F32, tag="s")
                        for q in range(tsz):
                            nc.tensor.matmul(
                                s_t[:, q, :],
                                xt[:, jbs[q] * P : (jbs[q] + 1) * P],
                                xt[:, n_lo : n_lo + 512],
                                start=True,
                                stop=True,
                            )
                        if kind == "save":
                            dst = sv[:, sv_off : sv_off + tsz, :]
                            rhss = [
                                sv[:, sv_off + q, :] for q in range(tsz)
                            ]
                            sv_off += tsz
                        else:
                            w_t = w_pool.tile([P, JT, 512], Y_MM_DT, tag="w")
                            dst = w_t[:, :tsz, :]
                            rhss = [w_t[:, q, :] for q in range(tsz)]
                        nc.scalar.activation(
                            dst,
                            s_t[:, :tsz, :],
                            mybir.ActivationFunctionType.Exp,
                            bias=negc,
                            scale=1.0,
                        )
                    # Y matmuls run two batches behind the S matmuls so PE
                    # always has independent S work queued when a group ends.
                    pending_y.append((jbs, rhss, flags, yt))
                    if len(pending_y) > 3:
                        emit_y(pending_y.pop(0))
                    walked += tsz
                    jt_idx += 1
                    if jt_idx == 3:
                        if pending_tail is not None:
                            pending_tail()
                            pending_tail = None
                        if g == G - 1:
                            # normalize finished groups while the last group
                            # is still computing (DVE-only, so this doesn't
                            # touch the busy ACT engine)
                            normalize_and_store(0, G - 1)

                pending_tail = make_tail(g, yt, fine=(g == G - 1))
                prev_sv = sv

            for py in pending_y:
                emit_y(py)
            pending_tail()
            normalize_and_store(G - 1, G)

    nc.compile()
    return nc


def _get_nc():
    global _CACHED_NC
    if _CACHED_NC is None:
        _CACHED_NC = _build()
    return _CACHED_NC


def kernel(X: np.ndarray, bandwidth: np.ndarray, **run_kwargs):
    """Full-input entry point: X [8, 4096, 128] f32, bandwidth scalar f32.

    Returns [8, 4096, 128] f32. Distributes one batch per NeuronCore.
    """
    X = np.ascontiguousarray(X, dtype=np.float32)
    B = X.shape[0]
    assert X.shape == (B, N, D), X.shape
    bw = np.asarray(bandwidth, dtype=np.float32).reshape(1)

    nc = _get_nc()
    in_maps = [{"X": X[b], "bandwidth": bw} for b in range(B)]
    try:
        res = run_bass_kernel_spmd(nc, in_maps, core_ids=list(range(B)), **run_kwargs)
    except Exception:
        # The first execution after other jax-on-neuron work occasionally hits
        # a transient NRT_EXEC_UNIT_UNRECOVERABLE; a retry succeeds.
        res = run_bass_kernel_spmd(nc, in_maps, core_ids=list(range(B)), **run_kwargs)
    out = np.stack([res.results[b]["Y"] for b in range(B)], axis=0)
    kernel.last_results = res
    return out


if __name__ == "__main__":
    rng = np.random.default_rng(0)
    X = rng.standard_normal((8, N, D), dtype=np.float32)
    X /= np.linalg.norm(X, axis=-1, keepdims=True)
    out = kernel(X=X, bandwidth=np.float32(0.1))
    print("out shape", out.shape, "finite", np.isfinite(out).all())



# revision 4
# speedup vs baseline: 1.2698x; 1.2698x over previous
"""Trainium2 Bass kernel for GBMS mean-shift step (nn_GBMS_RNN_137438953906).

Math (per batch b):
    W = exp((X X^T - 1) / bandwidth^2)          [N, N]
    Y = (W @ X) / rowsum(W)                     [N, D]
    out = Y / max(||Y||_2, 1e-12)  (L2 norm along D)

rowsum(W) is a positive per-row scalar, so it cancels in the final L2
normalization; we never compute row sums.  Uniform scales on X cancel the
same way, so X is carried as 8*X (fp8-friendly range, exact power of 2).

Sharding: data-parallel over batch B=8 across the 8 NeuronCores.

Per-core dataflow (N=4096 as 8 column stripes of 512; W tiles are
[128 j-rows x 512 stripe-cols], 32 j-blocks per stripe):
  xt8[d64, 2, n] = 8*X^T in fp8e4m3  (PE half-transposes of bf16 8X + DVE
      convert; the [64,2] split is the DoubleRow matmul's paired-K layout)
  direct tile (jb, g):  S = xt8_jb^T xt8_g   (fp8 DoubleRow, 0.5 cyc/row)
                        W = exp(S/(64 b^2) - 1/b^2) -> bf16
                        (ACT, 3-tile batches, runtime scale/bias APs)
  symmetry reuse: W is symmetric, so the 4g tiles of stripe g above the
      diagonal are never recomputed: when stripe g' finishes the 4-tile
      group destined for stripe gd, ONE wide XBAR DMA-transpose turns the
      group [128, 2048] into wr [128, 16, 128] whose strided views
      wr[:, q::4, :] are ready-to-use transposed rhs tiles for stripe gd.
      This removes 44% of the exp work (ACT is the co-bottleneck with PE)
      and 44% of the S matmuls, at zero PE/ACT cost (DMA+HWDGE are idle).
  Y accumulation: yt[d, n512] += x16_jb^T @ W_tile  (bf16 matmuls, PSUM).
  Tail per stripe: yt -> bf16 stage -> PE transpose -> y16[n, d]; squares
      + row-reduce on the otherwise idle Pool engine; fast-inverse-sqrt
      normalization (DVE bit trick + 2 Newton steps); f32 stores.

fp8/bf16 error budget (worst case b=1.0): fp8 X quantization perturbs the
exponent by ~4.5e-3 rms -> ~0.5% output; bf16 W and bf16 X add ~0.1% each.
At b=0.1 the diagonal dominates W and the output is bf16(x_n) exactly.
"""

import sys

if "/opt/trn_rl_repo" not in sys.path:
    sys.path.insert(0, "/opt/trn_rl_repo")

import numpy as np

import concourse.mybir as mybir
from concourse import bacc
from concourse.tile import TileContext
from concourse.bass_utils import run_bass_kernel_spmd
from concourse.masks import make_identity

P = 128
N = 4096
D = 128
NB = N // P  # 32 row blocks
G = N // 512  # 8 column stripes
NCHUNK = 8  # input DMA chunks (4 row-blocks each)

F32 = mybir.dt.float32
BF16 = mybir.dt.bfloat16
FP8 = mybir.dt.float8e4
I32 = mybir.dt.int32
DR = mybir.MatmulPerfMode.DoubleRow

_CACHED_NC = None


def _build():
    nc = bacc.Bacc("TRN2", target_bir_lowering=False, debug=False)

    x_in = nc.dram_tensor("X", [N, D], F32, kind="ExternalInput")
    bw_in = nc.dram_tensor("bandwidth", [1], F32, kind="ExternalInput")
    y_out = nc.dram_tensor("Y", [N, D], F32, kind="ExternalOutput")

    x_src = x_in.rearrange("(jb p) d -> p jb d", p=P)  # [128, 32, 128] view
    y_dst = y_out.rearrange("(nb p) d -> p nb d", p=P)

    with TileContext(nc) as tc:
        with (
            tc.tile_pool(name="const", bufs=1) as const,
            tc.tile_pool(name="bigf32", bufs=1) as bigf32,
            tc.tile_pool(name="svpool", bufs=2) as sv_pool,
            tc.tile_pool(name="wrpool", bufs=17) as wr_pool,
            tc.tile_pool(name="sqpool", bufs=2) as sq_pool,
            tc.tile_pool(name="stgpool", bufs=2) as stg_pool,
            tc.tile_pool(name="spsum", bufs=2, space="PSUM") as s_pool,
            tc.tile_pool(name="ytpsum", bufs=1, space="PSUM") as yt_pool,
            tc.tile_pool(name="tppsum", bufs=1, space="PSUM") as tp_pool,
        ):
            # ---- input DMAs: chunk 0 first (it gates the pipeline) ----
            x_nat = bigf32.tile([P, NB, D], F32, tag="big", name="x_nat")
            cb = NB // NCHUNK  # 4 row blocks per chunk
            nc.sync.dma_start(x_nat[:, 0:cb, :], x_src[:, 0:cb, :])

            bw = const.tile([P, 1], F32)
            nc.gpsimd.dma_start(bw[:], bw_in[None, :].to_broadcast([P, 1]))

            # identity built on the Pool engine (no DMA slot needed)
            ident = const.tile([P, P], F32)
            make_identity(nc, ident[:])
            identb = const.tile([P, P], BF16)
            nc.vector.tensor_copy(identb[:], ident[:])

            for c in range(1, NCHUNK):
                nc.sync.dma_start(
                    x_nat[:, c * cb : (c + 1) * cb, :],
                    x_src[:, c * cb : (c + 1) * cb, :],
                )

            # ---- runtime scalars ----
            scr = const.tile([P, 5], F32)
            bsq = scr[:, 0:1]
            rb2 = scr[:, 1:2]
            negc = scr[:, 2:3]
            sc64 = scr[:, 3:4]
            dummy = scr[:, 4:5]
            nc.vector.tensor_tensor(bsq, bw[:], bw[:], mybir.AluOpType.mult)
            nc.vector.reciprocal(rb2, bsq)  # 1/b^2
            nc.vector.tensor_scalar_mul(negc, rb2, -1.0)  # -1/b^2
            nc.vector.tensor_scalar_mul(sc64, rb2, 1.0 / 64.0)  # 1/(64 b^2)

            # preload the Exp ACT table while DMAs stream in
            nc.scalar.activation(dummy, bsq, mybir.ActivationFunctionType.Exp)

            x16 = const.tile([P, NB, D], BF16)  # 8*X, Y-matmul lhsT
            xt8 = const.tile([64, 2, N], FP8)  # 8*X^T, S-matmul operands

            # PE warm-up junk transposes (ramp the PE clock during DMA wait)
            warm = s_pool.tile([P, 3, 512], F32, tag="s", name="warm")
            for t in range(6):
                nc.tensor.transpose(
                    warm[:, t // 2, (t % 2) * P : (t % 2 + 1) * P],
                    ident[:],
                    ident[:],
                )

            chunks_done = [0]

            def emit_chunk(c):
                blk = slice(c * cb, (c + 1) * cb)
                nc.vector.tensor_scalar_mul(x16[:, blk, :], x_nat[:, blk, :], 8.0)
                xtp = tp_pool.tile([64, 2, 512], BF16, tag="tp", name="xtp")
                for o in range(cb):
                    jb = c * cb + o
                    for i in range(2):
                        nc.tensor.transpose(
                            xtp[:, i, o * P : (o + 1) * P],
                            x16[:, jb, i * 64 : (i + 1) * 64],
                            identb[:],
                        )
                nc.vector.tensor_copy(xt8[:, :, c * 512 : (c + 1) * 512], xtp[:])

            def need_chunks(upto):
                while chunks_done[0] <= min(upto, NCHUNK - 1):
                    emit_chunk(chunks_done[0])
                    chunks_done[0] += 1

            # ---- output staging ----
            y16 = const.tile([P, NB, D], BF16)  # [n_in_block, nb, d]
            ss_all = const.tile([P, NB], F32)
            half = const.tile([P, NB], F32)
            tmp = const.tile([P, NB], F32)
            rcp = const.tile([P, NB], F32)
            magic = const.tile([P, NB], I32)
            shreg = const.tile([P, NB], I32)
            nc.vector.memset(magic[:], 0x5F3759DF)
            y_stage_box = [None]

            def normalize_and_store(g0, g1):
                """L2-normalize output rows of stripes [g0, g1) and DMA out.
                1/norm = rsqrt(ss) via fast-inverse-sqrt + 2 Newton steps
                (DVE-only).  ss == 0 rows stay finite, matching the
                reference's eps-guarded division."""
                y_stage = y_stage_box[0]
                lo, hi = g0 * 4, g1 * 4  # nb range
                ss = ss_all[:, lo:hi]
                rs = rcp[:, lo:hi]
                hf = half[:, lo:hi]
                tm = tmp[:, lo:hi]
                nc.vector.tensor_scalar(
                    shreg[:, lo:hi],
                    ss.bitcast(I32),
                    1,
                    None,
                    mybir.AluOpType.logical_shift_right,
                )
                nc.vector.tensor_tensor(
                    rs.bitcast(I32),
                    magic[:, lo:hi],
                    shreg[:, lo:hi],
                    mybir.AluOpType.subtract,
                )
                nc.vector.tensor_scalar_mul(hf, ss, 0.5)
                for _ in range(2):
                    nc.vector.tensor_tensor(tm, rs, rs, mybir.AluOpType.mult)
                    nc.vector.tensor_tensor(tm, tm, hf, mybir.AluOpType.mult)
                    nc.vector.tensor_scalar(
                        tm, tm, -1.0, 1.5, mybir.AluOpType.mult, mybir.AluOpType.add
                    )
                    nc.vector.tensor_tensor(rs, rs, tm, mybir.AluOpType.mult)
                for nb in range(lo, hi):
                    nc.vector.tensor_scalar_mul(
                        y_stage[:, nb, :], y16[:, nb, :], rcp[:, nb : nb + 1]
                    )
                mid = (lo + hi) // 2
                nc.sync.dma_start(y_dst[:, lo:mid, :], y_stage[:, lo:mid, :])
                nc.sync.dma_start(y_dst[:, mid:hi, :], y_stage[:, mid:hi, :])

            def make_tail(g, stg, fine=False):
                """Tail of stripe g: stg (= yt in bf16) -> PE transpose ->
                y16[n, d]; squares + row reduce on the idle Pool engine."""

                def tail():
                    halves = 2 if fine else 1
                    hw_ = 4 // halves
                    for h in range(halves):
                        tp = tp_pool.tile([P, 4, P], BF16, tag="tp", name="tp")
                        for t in range(hw_):
                            tt = h * hw_ + t
                            nc.tensor.transpose(
                                tp[:, t, :],
                                stg[:, tt * P : (tt + 1) * P],
                                identb[:],
                            )
                        nbs = slice(g * 4 + h * hw_, g * 4 + (h + 1) * hw_)
                        nc.vector.tensor_copy(y16[:, nbs, :], tp[:, 0:hw_, :])
                    sqt = sq_pool.tile([P, 4, P], F32, tag="sq", name="sqt")
                    nbs = slice(g * 4, g * 4 + 4)
                    nc.gpsimd.tensor_tensor(
                        sqt[:], y16[:, nbs, :], y16[:, nbs, :], mybir.AluOpType.mult
                    )
                    nc.vector.tensor_reduce(
                        ss_all[:, nbs],
                        sqt[:],
                        axis=mybir.AxisListType.X,
                        op=mybir.AluOpType.add,
                    )

                return tail

            wr_tiles = {}  # (gs, gd) -> wide-transposed 4-tile group
            pending_tail = None

            # ---- main loop over column stripes ----
            for g in range(G):
                ndirect = 32 - 4 * g
                yt = yt_pool.tile([P, 512], F32, tag="yt", name="yt")
                sv = sv_pool.tile([P, 32, 512], BF16, tag="sv", name="sv")

                # direct j-blocks 4g..31 in exp batches of 3 (ragged last)
                batches = []
                s = 0
                while s < ndirect:
                    t = min(3, ndirect - s)
                    batches.append(list(range(s, s + t)))
                    s += t

                # Y-matmul emission bookkeeping: 32 per stripe, flags by
                # actual emission order into yt.
                n_emitted = [0]

                def emit_y(jb, rhs, yt=yt, n_emitted=n_emitted):
                    nc.tensor.matmul(
                        yt[:],
                        x16[:, jb, :],
                        rhs,
                        start=(n_emitted[0] == 0),
                        stop=(n_emitted[0] == 31),
                    )
                    n_emitted[0] += 1

                # reuse tiles (ready now, via wr views from earlier stripes)
                rq = []
                for jb in range(4 * g):
                    gs, q = jb // 4, jb % 4
                    rq.append((jb, wr_tiles[(gs, g)][:, q:16:4, :]))
                rpc = -(-len(rq) // len(batches))  # ceil: spread over cycles

                dq_ready = []  # direct (jb, rhs) whose exp has been emitted

                for k, slots in enumerate(batches):
                    if g == 0:
                        need_chunks(slots[-1] // cb + (1 if k > 0 else 0))
                    # S matmuls for batch k (fp8 DoubleRow)
                    s_t = s_pool.tile([P, 3, 512], F32, tag="s", name="s_t")
                    for q, sl in enumerate(slots):
                        jb = 4 * g + sl
                        nc.tensor.matmul(
                            s_t[:, q, :],
                            xt8[:, :, jb * P : (jb + 1) * P],
                            xt8[:, :, g * 512 : (g + 1) * 512],
                            start=True,
                            stop=True,
                            perf_mode=DR,
                        )
                    # exp batch k -> sv slots (bf16)
                    nc.scalar.activation(
                        sv[:, slots[0] : slots[-1] + 1, :],
                        s_t[:, 0 : len(slots), :],
                        mybir.ActivationFunctionType.Exp,
                        bias=negc,
                        scale=sc64,
                    )
                    for sl in slots:
                        dq_ready.append((4 * g + sl, sv[:, sl, :]))
                        # 4-tile group complete -> wide DMA transpose for
                        # the stripe it serves
                        if sl % 4 == 3 and sl >= 4:
                            gd = g + sl // 4
                            wr = wr_pool.tile(
                                [P, 16, P], BF16, tag="wr", name="wr"
                            )
                            wr_tiles[(g, gd)] = wr
                            nc.sync.dma_start_transpose(
                                wr[:],
                                sv[:, sl - 3 : sl + 1, :].rearrange(
                                    "p a b -> p (a b)"
                                ),
                            )
                    # reuse-Y fillers (no ACT dependency)
                    for _ in range(rpc):
                        if rq:
                            emit_y(*rq.pop(0))
                    # direct-Y, trailing one exp batch
                    while len(dq_ready) > len(slots):
                        emit_y(*dq_ready.pop(0))
                    # deferred work
                    if k == 1:
                        if pending_tail is not None:
                            pending_tail()
                            pending_tail = None
                        if g == G - 1:
                            y_stage_box[0] = bigf32.tile(
                                [P, NB, D], F32, tag="big", name="y_stage"
                            )
                            normalize_and_store(0, G - 1)

                for jb, rhs in rq:
                    emit_y(jb, rhs)
                for jb, rhs in dq_ready:
                    emit_y(jb, rhs)
                assert n_emitted[0] == 32, n_emitted[0]

                # stage yt out of PSUM immediately (yt pool has 1 buf)
                stg = stg_pool.tile([P, 512], BF16, tag="stg", name="stg")
                nc.vector.tensor_copy(stg[:], yt[:])
                pending_tail = make_tail(g, stg, fine=(g == G - 1))

            pending_tail()
            normalize_and_store(G - 1, G)

    nc.compile()
    return nc


def _get_nc():
    global _CACHED_NC
    if _CACHED_NC is None:
        _CACHED_NC = _build()
    return _CACHED_NC


def kernel(X: np.ndarray, bandwidth: np.ndarray, **run_kwargs):
    """Full-input entry point: X [8, 4096, 128] f32, bandwidth scalar f32.

    Returns [8, 4096, 128] f32. Distributes one batch per NeuronCore.
    """
    X = np.ascontiguousarray(X, dtype=np.float32)
    B = X.shape[0]
    assert X.shape == (B, N, D), X.shape
    bw = np.asarray(bandwidth, dtype=np.float32).reshape(1)

    nc = _get_nc()
    in_maps = [{"X": X[b], "bandwidth": bw} for b in range(B)]
    try:
        res = run_bass_kernel_spmd(nc, in_maps, core_ids=list(range(B)), **run_kwargs)
    except Exception:
        # The first execution after other jax-on-neuron work occasionally hits
        # a transient NRT_EXEC_UNIT_UNRECOVERABLE; a retry succeeds.
        res = run_bass_kernel_spmd(nc, in_maps, core_ids=list(range(B)), **run_kwargs)
    out = np.stack([res.results[b]["Y"] for b in range(B)], axis=0)
    kernel.last_results = res
    return out


if __name__ == "__main__":
    rng = np.random.default_rng(0)
    X = rng.standard_normal((8, N, D), dtype=np.float32)
    X /= np.linalg.norm(X, axis=-1, keepdims=True)
    out = kernel(X=X, bandwidth=np.float32(0.1))
    print("out shape", out.shape, "finite", np.isfinite(out).all())


# revision 5
# speedup vs baseline: 1.3048x; 1.0276x over previous
"""Trainium2 Bass kernel for GBMS mean-shift step (nn_GBMS_RNN_137438953906).

Math (per batch b):
    W = exp((X X^T - 1) / bandwidth^2)          [N, N]
    Y = (W @ X) / rowsum(W)                     [N, D]
    out = Y / max(||Y||_2, 1e-12)  (L2 norm along D)

rowsum(W) is a positive per-row scalar, so it cancels in the final L2
normalization; we never compute row sums.  Uniform scales on X cancel the
same way, so X is carried as 8*X (fp8-friendly range, exact power of 2).

Sharding: data-parallel over batch B=8 across the 8 NeuronCores.

Per-core dataflow (N=4096 as 8 column stripes of 512; W tiles are
[128 j-rows x 512 stripe-cols], 32 j-blocks per stripe):
  xt8[d64, 2, n] = 8*X^T in fp8e4m3  (PE half-transposes of bf16 8X + DVE
      convert; the [64,2] split is the DoubleRow matmul's paired-K layout)
  direct tile (jb, g):  S = xt8_jb^T xt8_g   (fp8 DoubleRow, 0.5 cyc/row)
                        W = exp(S/(64 b^2) - 1/b^2) -> bf16
                        (ACT, 3-tile batches, runtime scale/bias APs)
  symmetry reuse: W is symmetric, so the 4g tiles of stripe g above the
      diagonal are never recomputed: when stripe g' finishes the 4-tile
      group destined for stripe gd, ONE wide XBAR DMA-transpose turns the
      group [128, 2048] into wr [128, 16, 128] whose strided views
      wr[:, q::4, :] are ready-to-use transposed rhs tiles for stripe gd.
      This removes 44% of the exp work (ACT is the co-bottleneck with PE)
      and 44% of the S matmuls, at zero PE/ACT cost (DMA+HWDGE are idle).
  Y accumulation: yt[d, n512] += x16_jb^T @ W_tile  (bf16 matmuls, PSUM).
  Tail per stripe: yt -> bf16 stage -> PE transpose -> y16[n, d]; squares
      + row-reduce on the otherwise idle Pool engine; fast-inverse-sqrt
      normalization (DVE bit trick + 2 Newton steps); f32 stores.

fp8/bf16 error budget (worst case b=1.0): fp8 X quantization perturbs the
exponent by ~4.5e-3 rms -> ~0.5% output; bf16 W and bf16 X add ~0.1% each.
At b=0.1 the diagonal dominates W and the output is bf16(x_n) exactly.
"""

import sys

if "/opt/trn_rl_repo" not in sys.path:
    sys.path.insert(0, "/opt/trn_rl_repo")

import numpy as np

import concourse.mybir as mybir
from concourse import bacc
from concourse.tile import TileContext
from concourse.bass_utils import run_bass_kernel_spmd
from concourse.masks import make_identity

P = 128
N = 4096
D = 128
NB = N // P  # 32 row blocks
G = N // 512  # 8 column stripes
NCHUNK = 8  # input DMA chunks (4 row-blocks each)

F32 = mybir.dt.float32
BF16 = mybir.dt.bfloat16
FP8 = mybir.dt.float8e4
I32 = mybir.dt.int32
DR = mybir.MatmulPerfMode.DoubleRow

_CACHED_NC = None


def _build():
    nc = bacc.Bacc("TRN2", target_bir_lowering=False, debug=False)

    x_in = nc.dram_tensor("X", [N, D], F32, kind="ExternalInput")
    bw_in = nc.dram_tensor("bandwidth", [1], F32, kind="ExternalInput")
    y_out = nc.dram_tensor("Y", [N, D], F32, kind="ExternalOutput")

    x_src = x_in.rearrange("(jb p) d -> p jb d", p=P)  # [128, 32, 128] view
    y_dst = y_out.rearrange("(nb p) d -> p nb d", p=P)

    with TileContext(nc) as tc:
        with (
            tc.tile_pool(name="const", bufs=1) as const,
            tc.tile_pool(name="bigf32", bufs=1) as bigf32,
            tc.tile_pool(name="svpool", bufs=2) as sv_pool,
            tc.tile_pool(name="wrpool", bufs=17) as wr_pool,
            tc.tile_pool(name="sqpool", bufs=2) as sq_pool,
            tc.tile_pool(name="stgpool", bufs=2) as stg_pool,
            tc.tile_pool(name="spsum", bufs=2, space="PSUM") as s_pool,
            tc.tile_pool(name="ytpsum", bufs=1, space="PSUM") as yt_pool,
            tc.tile_pool(name="tppsum", bufs=1, space="PSUM") as tp_pool,
        ):
            # ---- input DMAs: chunk 0 first (it gates the pipeline) ----
            x_nat = bigf32.tile([P, NB, D], F32, tag="big", name="x_nat")
            cb = NB // NCHUNK  # 4 row blocks per chunk
            nc.sync.dma_start(x_nat[:, 0:cb, :], x_src[:, 0:cb, :])

            bw = const.tile([P, 1], F32)
            nc.gpsimd.dma_start(bw[:], bw_in[None, :].to_broadcast([P, 1]))

            # identity built on the Pool engine (no DMA slot needed)
            ident = const.tile([P, P], F32)
            make_identity(nc, ident[:])
            identb = const.tile([P, P], BF16)
            nc.vector.tensor_copy(identb[:], ident[:])

            for c in range(1, NCHUNK):
                nc.sync.dma_start(
                    x_nat[:, c * cb : (c + 1) * cb, :],
                    x_src[:, c * cb : (c + 1) * cb, :],
                )

            # ---- runtime scalars ----
            scr = const.tile([P, 5], F32)
            bsq = scr[:, 0:1]
            rb2 = scr[:, 1:2]
            negc = scr[:, 2:3]
            sc64 = scr[:, 3:4]
            dummy = scr[:, 4:5]
            nc.vector.tensor_tensor(bsq, bw[:], bw[:], mybir.AluOpType.mult)
            nc.vector.reciprocal(rb2, bsq)  # 1/b^2
            nc.vector.tensor_scalar_mul(negc, rb2, -1.0)  # -1/b^2
            nc.vector.tensor_scalar_mul(sc64, rb2, 1.0 / 64.0)  # 1/(64 b^2)

            # preload the Exp ACT table while DMAs stream in
            nc.scalar.activation(dummy, bsq, mybir.ActivationFunctionType.Exp)

            x16 = const.tile([P, NB, D], BF16)  # 8*X, Y-matmul lhsT
            xt8 = const.tile([64, 2, N], FP8)  # 8*X^T, S-matmul operands

            # PE warm-up junk transposes (ramp the PE clock during DMA wait)
            warm = s_pool.tile([P, 3, 512], F32, tag="s", name="warm")
            for t in range(6):
                nc.tensor.transpose(
                    warm[:, t // 2, (t % 2) * P : (t % 2 + 1) * P],
                    ident[:],
                    ident[:],
                )

            chunks_done = [0]

            def emit_chunk(c):
                blk = slice(c * cb, (c + 1) * cb)
                nc.vector.tensor_scalar_mul(x16[:, blk, :], x_nat[:, blk, :], 8.0)
                xtp = tp_pool.tile([64, 2, 512], BF16, tag="tp", name="xtp")
                for o in range(cb):
                    jb = c * cb + o
                    for i in range(2):
                        nc.tensor.transpose(
                            xtp[:, i, o * P : (o + 1) * P],
                            x16[:, jb, i * 64 : (i + 1) * 64],
                            identb[:],
                        )
                nc.vector.tensor_copy(xt8[:, :, c * 512 : (c + 1) * 512], xtp[:])

            def need_chunks(upto):
                while chunks_done[0] <= min(upto, NCHUNK - 1):
                    emit_chunk(chunks_done[0])
                    chunks_done[0] += 1

            # ---- output staging ----
            y16 = const.tile([P, NB, D], BF16)  # [n_in_block, nb, d]
            ss_all = const.tile([P, NB], F32)
            half = const.tile([P, NB], F32)
            tmp = const.tile([P, NB], F32)
            rcp = const.tile([P, NB], F32)
            magic = const.tile([P, NB], I32)
            shreg = const.tile([P, NB], I32)
            nc.vector.memset(magic[:], 0x5F3759DF)
            y_stage_box = [None]

            def normalize_and_store(g0, g1):
                """L2-normalize output rows of stripes [g0, g1) and DMA out.
                1/norm = rsqrt(ss) via fast-inverse-sqrt + 2 Newton steps
                (DVE-only).  ss == 0 rows stay finite, matching the
                reference's eps-guarded division."""
                y_stage = y_stage_box[0]
                lo, hi = g0 * 4, g1 * 4  # nb range
                ss = ss_all[:, lo:hi]
                rs = rcp[:, lo:hi]
                hf = half[:, lo:hi]
                tm = tmp[:, lo:hi]
                nc.vector.tensor_scalar(
                    shreg[:, lo:hi],
                    ss.bitcast(I32),
                    1,
                    None,
                    mybir.AluOpType.logical_shift_right,
                )
                nc.vector.tensor_tensor(
                    rs.bitcast(I32),
                    magic[:, lo:hi],
                    shreg[:, lo:hi],
                    mybir.AluOpType.subtract,
                )
                nc.vector.tensor_scalar_mul(hf, ss, 0.5)
                for _ in range(2):
                    nc.vector.tensor_tensor(tm, rs, rs, mybir.AluOpType.mult)
                    nc.vector.tensor_tensor(tm, tm, hf, mybir.AluOpType.mult)
                    nc.vector.tensor_scalar(
                        tm, tm, -1.0, 1.5, mybir.AluOpType.mult, mybir.AluOpType.add
                    )
                    nc.vector.tensor_tensor(rs, rs, tm, mybir.AluOpType.mult)
                for nb in range(lo, hi):
                    nc.vector.tensor_scalar_mul(
                        y_stage[:, nb, :], y16[:, nb, :], rcp[:, nb : nb + 1]
                    )
                mid = (lo + hi) // 2
                nc.sync.dma_start(y_dst[:, lo:mid, :], y_stage[:, lo:mid, :])
                nc.sync.dma_start(y_dst[:, mid:hi, :], y_stage[:, mid:hi, :])

            def make_tail(g, stg, fine=False):
                """Tail of stripe g: stg (= yt in bf16) -> PE transpose ->
                y16[n, d]; squares + row reduce on the idle Pool engine."""

                def tail():
                    halves = 2 if fine else 1
                    hw_ = 4 // halves
                    for h in range(halves):
                        tp = tp_pool.tile([P, 4, P], BF16, tag="tp", name="tp")
                        for t in range(hw_):
                            tt = h * hw_ + t
                            nc.tensor.transpose(
                                tp[:, t, :],
                                stg[:, tt * P : (tt + 1) * P],
                                identb[:],
                            )
                        nbs = slice(g * 4 + h * hw_, g * 4 + (h + 1) * hw_)
                        nc.vector.tensor_copy(y16[:, nbs, :], tp[:, 0:hw_, :])
                    sqt = sq_pool.tile([P, 4, P], F32, tag="sq", name="sqt")
                    nbs = slice(g * 4, g * 4 + 4)
                    nc.gpsimd.tensor_tensor(
                        sqt[:], y16[:, nbs, :], y16[:, nbs, :], mybir.AluOpType.mult
                    )
                    nc.vector.tensor_reduce(
                        ss_all[:, nbs],
                        sqt[:],
                        axis=mybir.AxisListType.X,
                        op=mybir.AluOpType.add,
                    )

                return tail

            wr_tiles = {}  # (gs, gd) -> wide-transposed 4-tile group
            pending = {"flush": None, "tail": None}

            def make_flush(g, rq, dq_ready, emit_y, yt, n_emitted):
                def flush():
                    for jb, rhs in rq:
                        emit_y(jb, rhs)
                    for jb, rhs in dq_ready:
                        emit_y(jb, rhs)
                    assert n_emitted[0] == 32, n_emitted[0]
                    stg = stg_pool.tile([P, 512], BF16, tag="stg", name="stg")
                    nc.vector.tensor_copy(stg[:], yt[:])
                    pending["tail"] = make_tail(g, stg, fine=(g == G - 1))

                return flush

            # ---- main loop over column stripes ----
            for g in range(G):
                ndirect = 32 - 4 * g
                yt = yt_pool.tile([P, 512], F32, tag="yt", name="yt")
                sv = sv_pool.tile([P, 32, 512], BF16, tag="sv", name="sv")

                # direct j-blocks 4g..31 in exp batches of 3 (ragged last)
                batches = []
                s = 0
                while s < ndirect:
                    t = min(3, ndirect - s)
                    batches.append(list(range(s, s + t)))
                    s += t

                # Y-matmul emission bookkeeping: 32 per stripe, flags by
                # actual emission order into yt.
                n_emitted = [0]

                def emit_y(jb, rhs, yt=yt, n_emitted=n_emitted):
                    nc.tensor.matmul(
                        yt[:],
                        x16[:, jb, :],
                        rhs,
                        start=(n_emitted[0] == 0),
                        stop=(n_emitted[0] == 31),
                    )
                    n_emitted[0] += 1

                # reuse tiles (ready now, via wr views from earlier stripes)
                rq = []
                for jb in range(4 * g):
                    gs, q = jb // 4, jb % 4
                    rq.append((jb, wr_tiles[(gs, g)][:, q:16:4, :]))
                rpc = -(-len(rq) // len(batches))  # ceil: spread over cycles

                dq_ready = []  # direct (jb, rhs) whose exp has been emitted

                for k, slots in enumerate(batches):
                    if g == 0:
                        need_chunks(slots[-1] // cb + (1 if k > 0 else 0))
                    # S matmuls for batch k (fp8 DoubleRow)
                    s_t = s_pool.tile([P, 3, 512], F32, tag="s", name="s_t")
                    for q, sl in enumerate(slots):
                        jb = 4 * g + sl
                        nc.tensor.matmul(
                            s_t[:, q, :],
                            xt8[:, :, jb * P : (jb + 1) * P],
                            xt8[:, :, g * 512 : (g + 1) * 512],
                            start=True,
                            stop=True,
                            perf_mode=DR,
                        )
                    # exp batch k -> sv slots (bf16)
                    nc.scalar.activation(
                        sv[:, slots[0] : slots[-1] + 1, :],
                        s_t[:, 0 : len(slots), :],
                        mybir.ActivationFunctionType.Exp,
                        bias=negc,
                        scale=sc64,
                    )
                    for sl in slots:
                        dq_ready.append((4 * g + sl, sv[:, sl, :]))
                        # 4-tile group complete -> wide DMA transpose for
                        # the stripe it serves
                        if sl % 4 == 3 and sl >= 4:
                            gd = g + sl // 4
                            wr = wr_pool.tile(
                                [P, 16, P], BF16, tag="wr", name="wr"
                            )
                            wr_tiles[(g, gd)] = wr
                            nc.sync.dma_start_transpose(
                                wr[:],
                                sv[:, sl - 3 : sl + 1, :].rearrange(
                                    "p a b -> p (a b)"
                                ),
                            )
                    # previous stripe's trailing work goes AFTER this
                    # stripe's first S+exp so ACT never starves at the
                    # stripe boundary
                    if k == 0 and pending["flush"] is not None:
                        pending["flush"]()
                        pending["flush"] = None
                    # reuse-Y fillers (no ACT dependency)
                    for _ in range(rpc):
                        if rq:
                            emit_y(*rq.pop(0))
                    # direct-Y, trailing one exp batch
                    while len(dq_ready) > len(slots):
                        emit_y(*dq_ready.pop(0))
                    # deferred work: tails + normalization spread over the
                    # back stripes so the DVE/store burst overlaps PE work
                    if k == 1 and pending["tail"] is not None:
                        pending["tail"]()
                        pending["tail"] = None
                    if g == G - 2 and k == 2:
                        y_stage_box[0] = bigf32.tile(
                            [P, NB, D], F32, tag="big", name="y_stage"
                        )
                        normalize_and_store(0, 3)
                    if g == G - 1 and k == 0:
                        normalize_and_store(3, 5)
                    if g == G - 1 and k == 1:
                        normalize_and_store(5, G - 1)

                pending["flush"] = make_flush(g, rq, dq_ready, emit_y, yt, n_emitted)

            pending["flush"]()
            pending["tail"]()
            normalize_and_store(G - 1, G)

    nc.compile()
    return nc


def _get_nc():
    global _CACHED_NC
    if _CACHED_NC is None:
        _CACHED_NC = _build()
    return _CACHED_NC


def kernel(X: np.ndarray, bandwidth: np.ndarray, **run_kwargs):
    """Full-input entry point: X [8, 4096, 128] f32, bandwidth scalar f32.

    Returns [8, 4096, 128] f32. Distributes one batch per NeuronCore.
    """
    X = np.ascontiguousarray(X, dtype=np.float32)
    B = X.shape[0]
    assert X.shape == (B, N, D), X.shape
    bw = np.asarray(bandwidth, dtype=np.float32).reshape(1)

    nc = _get_nc()
    in_maps = [{"X": X[b], "bandwidth": bw} for b in range(B)]
    try:
        res = run_bass_kernel_spmd(nc, in_maps, core_ids=list(range(B)), **run_kwargs)
    except Exception:
        # The first execution after other jax-on-neuron work occasionally hits
        # a transient NRT_EXEC_UNIT_UNRECOVERABLE; a retry succeeds.
        res = run_bass_kernel_spmd(nc, in_maps, core_ids=list(range(B)), **run_kwargs)
    out = np.stack([res.results[b]["Y"] for b in range(B)], axis=0)
    kernel.last_results = res
    return out


if __name__ == "__main__":
    rng = np.random.default_rng(0)
    X = rng.standard_normal((8, N, D), dtype=np.float32)
    X /= np.linalg.norm(X, axis=-1, keepdims=True)
    out = kernel(X=X, bandwidth=np.float32(0.1))
    print("out shape", out.shape, "finite", np.isfinite(out).all())


# revision 6
# speedup vs baseline: 1.3068x; 1.0015x over previous
"""Trainium2 Bass kernel for GBMS mean-shift step (nn_GBMS_RNN_137438953906).

Math (per batch b):
    W = exp((X X^T - 1) / bandwidth^2)          [N, N]
    Y = (W @ X) / rowsum(W)                     [N, D]
    out = Y / max(||Y||_2, 1e-12)  (L2 norm along D)

rowsum(W) is a positive per-row scalar, so it cancels in the final L2
normalization; we never compute row sums.  Uniform scales on X cancel the
same way, so X is carried as 8*X (fp8-friendly range, exact power of 2).

Sharding: data-parallel over batch B=8 across the 8 NeuronCores.

Per-core dataflow (N=4096 as 8 column stripes of 512; W tiles are
[128 j-rows x 512 stripe-cols], 32 j-blocks per stripe):
  xt8[d64, 2, n] = 8*X^T in fp8e4m3  (PE half-transposes of bf16 8X + DVE
      convert; the [64,2] split is the DoubleRow matmul's paired-K layout)
  direct tile (jb, g):  S = xt8_jb^T xt8_g   (fp8 DoubleRow, 0.5 cyc/row)
                        W = exp(S/(64 b^2) - 1/b^2) -> bf16
                        (ACT, 3-tile batches, runtime scale/bias APs)
  symmetry reuse: W is symmetric, so the 4g tiles of stripe g above the
      diagonal are never recomputed: when stripe g' finishes the 4-tile
      group destined for stripe gd, ONE wide XBAR DMA-transpose turns the
      group [128, 2048] into wr [128, 16, 128] whose strided views
      wr[:, q::4, :] are ready-to-use transposed rhs tiles for stripe gd.
      This removes 44% of the exp work (ACT is the co-bottleneck with PE)
      and 44% of the S matmuls, at zero PE/ACT cost (DMA+HWDGE are idle).
  Y accumulation: yt[d, n512] += x16_jb^T @ W_tile  (bf16 matmuls, PSUM).
  Tail per stripe: yt -> bf16 stage -> PE transpose -> y16[n, d]; squares
      + row-reduce on the otherwise idle Pool engine; fast-inverse-sqrt
      normalization (DVE bit trick + 2 Newton steps); f32 stores.

fp8/bf16 error budget (worst case b=1.0): fp8 X quantization perturbs the
exponent by ~4.5e-3 rms -> ~0.5% output; bf16 W and bf16 X add ~0.1% each.
At b=0.1 the diagonal dominates W and the output is bf16(x_n) exactly.
"""

import sys

if "/opt/trn_rl_repo" not in sys.path:
    sys.path.insert(0, "/opt/trn_rl_repo")

import numpy as np

import concourse.mybir as mybir
from concourse import bacc
from concourse.tile import TileContext
from concourse.bass_utils import run_bass_kernel_spmd
from concourse.masks import make_identity

P = 128
N = 4096
D = 128
NB = N // P  # 32 row blocks
G = N // 512  # 8 column stripes
NCHUNK = 8  # input DMA chunks (4 row-blocks each)

F32 = mybir.dt.float32
BF16 = mybir.dt.bfloat16
FP8 = mybir.dt.float8e4
I32 = mybir.dt.int32
DR = mybir.MatmulPerfMode.DoubleRow

_CACHED_NC = None


def _build():
    nc = bacc.Bacc("TRN2", target_bir_lowering=False, debug=False)

    x_in = nc.dram_tensor("X", [N, D], F32, kind="ExternalInput")
    bw_in = nc.dram_tensor("bandwidth", [1], F32, kind="ExternalInput")
    y_out = nc.dram_tensor("Y", [N, D], F32, kind="ExternalOutput")

    x_src = x_in.rearrange("(jb p) d -> p jb d", p=P)  # [128, 32, 128] view
    y_dst = y_out.rearrange("(nb p) d -> p nb d", p=P)

    with TileContext(nc) as tc:
        with (
            tc.tile_pool(name="const", bufs=1) as const,
            tc.tile_pool(name="bigf32", bufs=1) as bigf32,
            tc.tile_pool(name="svpool", bufs=2) as sv_pool,
            tc.tile_pool(name="wrpool", bufs=17) as wr_pool,
            tc.tile_pool(name="sqpool", bufs=2) as sq_pool,
            tc.tile_pool(name="stgpool", bufs=2) as stg_pool,
            tc.tile_pool(name="spsum", bufs=2, space="PSUM") as s_pool,
            tc.tile_pool(name="ytpsum", bufs=1, space="PSUM") as yt_pool,
            tc.tile_pool(name="tppsum", bufs=1, space="PSUM") as tp_pool,
        ):
            # ---- input DMAs: chunk 0 first (it gates the pipeline) ----
            x_nat = bigf32.tile([P, NB, D], F32, tag="big", name="x_nat")
            cb = NB // NCHUNK  # 4 row blocks per chunk
            nc.sync.dma_start(x_nat[:, 0:cb, :], x_src[:, 0:cb, :])

            bw = const.tile([P, 1], F32)
            nc.gpsimd.dma_start(bw[:], bw_in[None, :].to_broadcast([P, 1]))

            # identity built on the Pool engine (no DMA slot needed)
            ident = const.tile([P, P], F32)
            make_identity(nc, ident[:])
            identb = const.tile([P, P], BF16)
            nc.vector.tensor_copy(identb[:], ident[:])

            for c in range(1, NCHUNK):
                nc.sync.dma_start(
                    x_nat[:, c * cb : (c + 1) * cb, :],
                    x_src[:, c * cb : (c + 1) * cb, :],
                )

            # ---- runtime scalars ----
            scr = const.tile([P, 5], F32)
            bsq = scr[:, 0:1]
            rb2 = scr[:, 1:2]
            negc = scr[:, 2:3]
            sc64 = scr[:, 3:4]
            dummy = scr[:, 4:5]
            nc.vector.tensor_tensor(bsq, bw[:], bw[:], mybir.AluOpType.mult)
            nc.vector.reciprocal(rb2, bsq)  # 1/b^2
            nc.vector.tensor_scalar_mul(negc, rb2, -1.0)  # -1/b^2
            nc.vector.tensor_scalar_mul(sc64, rb2, 1.0 / 64.0)  # 1/(64 b^2)

            # preload the Exp ACT table while DMAs stream in
            nc.scalar.activation(dummy, bsq, mybir.ActivationFunctionType.Exp)

            x16 = const.tile([P, NB, D], BF16)  # 8*X, Y-matmul lhsT
            xt8 = const.tile([64, 2, N], FP8)  # 8*X^T, S-matmul operands

            # PE warm-up junk transposes (ramp the PE clock during DMA wait)
            warm = s_pool.tile([P, 3, 512], F32, tag="s", name="warm")
            for t in range(6):
                nc.tensor.transpose(
                    warm[:, t // 2, (t % 2) * P : (t % 2 + 1) * P],
                    ident[:],
                    ident[:],
                )

            chunks_done = [0]

            def emit_chunk(c):
                blk = slice(c * cb, (c + 1) * cb)
                nc.vector.tensor_scalar_mul(x16[:, blk, :], x_nat[:, blk, :], 8.0)
                pool = tp_pool if c % 2 == 0 else yt_pool
                xtp = pool.tile(
                    [64, 2, 512], BF16, tag="tp" if c % 2 == 0 else "yt", name="xtp"
                )
                for o in range(cb):
                    jb = c * cb + o
                    for i in range(2):
                        nc.tensor.transpose(
                            xtp[:, i, o * P : (o + 1) * P],
                            x16[:, jb, i * 64 : (i + 1) * 64],
                            identb[:],
                        )
                nc.vector.tensor_copy(xt8[:, :, c * 512 : (c + 1) * 512], xtp[:])

            def need_chunks(upto):
                while chunks_done[0] <= min(upto, NCHUNK - 1):
                    emit_chunk(chunks_done[0])
                    chunks_done[0] += 1

            # ---- output staging ----
            y16 = const.tile([P, NB, D], BF16)  # [n_in_block, nb, d]
            ss_all = const.tile([P, NB], F32)
            half = const.tile([P, NB], F32)
            tmp = const.tile([P, NB], F32)
            rcp = const.tile([P, NB], F32)
            magic = const.tile([P, NB], I32)
            shreg = const.tile([P, NB], I32)
            nc.vector.memset(magic[:], 0x5F3759DF)
            y_stage_box = [None]

            def normalize_and_store(g0, g1):
                """L2-normalize output rows of stripes [g0, g1) and DMA out.
                1/norm = rsqrt(ss) via fast-inverse-sqrt + 2 Newton steps
                (DVE-only).  ss == 0 rows stay finite, matching the
                reference's eps-guarded division."""
                y_stage = y_stage_box[0]
                lo, hi = g0 * 4, g1 * 4  # nb range
                ss = ss_all[:, lo:hi]
                rs = rcp[:, lo:hi]
                hf = half[:, lo:hi]
                tm = tmp[:, lo:hi]
                nc.vector.tensor_scalar(
                    shreg[:, lo:hi],
                    ss.bitcast(I32),
                    1,
                    None,
                    mybir.AluOpType.logical_shift_right,
                )
                nc.vector.tensor_tensor(
                    rs.bitcast(I32),
                    magic[:, lo:hi],
                    shreg[:, lo:hi],
                    mybir.AluOpType.subtract,
                )
                nc.vector.tensor_scalar_mul(hf, ss, 0.5)
                for _ in range(2):
                    nc.vector.tensor_tensor(tm, rs, rs, mybir.AluOpType.mult)
                    nc.vector.tensor_tensor(tm, tm, hf, mybir.AluOpType.mult)
                    nc.vector.tensor_scalar(
                        tm, tm, -1.0, 1.5, mybir.AluOpType.mult, mybir.AluOpType.add
                    )
                    nc.vector.tensor_tensor(rs, rs, tm, mybir.AluOpType.mult)
                for nb in range(lo, hi):
                    nc.vector.tensor_scalar_mul(
                        y_stage[:, nb, :], y16[:, nb, :], rcp[:, nb : nb + 1]
                    )
                mid = (lo + hi) // 2
                nc.sync.dma_start(y_dst[:, lo:mid, :], y_stage[:, lo:mid, :])
                nc.sync.dma_start(y_dst[:, mid:hi, :], y_stage[:, mid:hi, :])

            def make_tail(g, stg, fine=False):
                """Tail of stripe g: stg (= yt in bf16) -> PE transpose ->
                y16[n, d]; squares on the idle Pool engine (all-DVE per-half
                pipeline in fine mode for the end-of-kernel critical path)."""

                def tail():
                    halves = 2 if fine else 1
                    hw_ = 4 // halves
                    for h in range(halves):
                        tp = tp_pool.tile([P, 4, P], BF16, tag="tp", name="tp")
                        for t in range(hw_):
                            tt = h * hw_ + t
                            nc.tensor.transpose(
                                tp[:, t, :],
                                stg[:, tt * P : (tt + 1) * P],
                                identb[:],
                            )
                        nbs = slice(g * 4 + h * hw_, g * 4 + (h + 1) * hw_)
                        nc.vector.tensor_copy(y16[:, nbs, :], tp[:, 0:hw_, :])
                        if fine:
                            sqt = sq_pool.tile([P, 4, P], F32, tag="sq", name="sqt")
                            nc.vector.tensor_tensor(
                                sqt[:, 0:hw_, :],
                                y16[:, nbs, :],
                                y16[:, nbs, :],
                                mybir.AluOpType.mult,
                            )
                            nc.vector.tensor_reduce(
                                ss_all[:, nbs],
                                sqt[:, 0:hw_, :],
                                axis=mybir.AxisListType.X,
                                op=mybir.AluOpType.add,
                            )
                    if not fine:
                        sqt = sq_pool.tile([P, 4, P], F32, tag="sq", name="sqt")
                        nbs = slice(g * 4, g * 4 + 4)
                        nc.gpsimd.tensor_tensor(
                            sqt[:], y16[:, nbs, :], y16[:, nbs, :],
                            mybir.AluOpType.mult,
                        )
                        nc.vector.tensor_reduce(
                            ss_all[:, nbs],
                            sqt[:],
                            axis=mybir.AxisListType.X,
                            op=mybir.AluOpType.add,
                        )

                return tail

            wr_tiles = {}  # (gs, gd) -> wide-transposed 4-tile group
            # cross-stripe pipeline state: leftover Y matmuls of the previous
            # stripe (drained a few per cycle so the next stripe's S/exp are
            # never stuck behind a matmul burst), then its stg copy + tail.
            state = {"carry": [], "stg": None, "tail": None}
            CR = 8  # carry drain rate per cycle

            # ---- main loop over column stripes ----
            for g in range(G):
                ndirect = 32 - 4 * g

                # direct j-blocks 4g..31 in exp batches of 3 (ragged last)
                batches = []
                s = 0
                while s < ndirect:
                    t = min(3, ndirect - s)
                    batches.append(list(range(s, s + t)))
                    s += t

                sv = sv_pool.tile([P, 32, 512], BF16, tag="sv", name="sv")
                yt_box = [None]
                n_emitted = [0]

                def emit_y(jb, rhs, yt_box=yt_box, n_emitted=n_emitted):
                    if yt_box[0] is None:
                        # lazy: the yt slot is shared with the odd xt chunks
                        # at startup and with the previous stripe's yt (whose
                        # stg copy must be emitted first)
                        yt_box[0] = yt_pool.tile(
                            [P, 512], F32, tag="yt", name="yt"
                        )
                    nc.tensor.matmul(
                        yt_box[0][:],
                        x16[:, jb, :],
                        rhs,
                        start=(n_emitted[0] == 0),
                        stop=(n_emitted[0] == 31),
                    )
                    n_emitted[0] += 1

                rq = []
                for jb in range(4 * g):
                    gs, q = jb // 4, jb % 4
                    rq.append((jb, wr_tiles[(gs, g)][:, q:16:4, :]))
                rpc = -(-len(rq) // len(batches))  # ceil: spread over cycles

                dq_ready = []  # direct (jb, rhs) whose exp has been emitted

                for k, slots in enumerate(batches):
                    if g == 0:
                        need_chunks(min(k + 1, NCHUNK - 1))
                    # S matmuls for batch k (fp8 DoubleRow)
                    s_t = s_pool.tile([P, 3, 512], F32, tag="s", name="s_t")
                    for q, sl in enumerate(slots):
                        jb = 4 * g + sl
                        nc.tensor.matmul(
                            s_t[:, q, :],
                            xt8[:, :, jb * P : (jb + 1) * P],
                            xt8[:, :, g * 512 : (g + 1) * 512],
                            start=True,
                            stop=True,
                            perf_mode=DR,
                        )
                    # exp batch k -> sv slots (bf16)
                    nc.scalar.activation(
                        sv[:, slots[0] : slots[-1] + 1, :],
                        s_t[:, 0 : len(slots), :],
                        mybir.ActivationFunctionType.Exp,
                        bias=negc,
                        scale=sc64,
                    )
                    for sl in slots:
                        dq_ready.append((4 * g + sl, sv[:, sl, :]))
                        # 4-tile group complete -> wide DMA transpose for
                        # the stripe it serves
                        if sl % 4 == 3 and sl >= 4:
                            gd = g + sl // 4
                            wr = wr_pool.tile(
                                [P, 16, P], BF16, tag="wr", name="wr"
                            )
                            wr_tiles[(g, gd)] = wr
                            nc.sync.dma_start_transpose(
                                wr[:],
                                sv[:, sl - 3 : sl + 1, :].rearrange(
                                    "p a b -> p (a b)"
                                ),
                            )
                    # drain the previous stripe's leftovers, then its stg
                    # copy + tail (frees the shared yt slot for this stripe)
                    for _ in range(CR):
                        if state["carry"]:
                            state["carry"].pop(0)()
                    if not state["carry"] and state["stg"] is not None:
                        state["stg"]()
                        state["stg"] = None
                        state["tail"]()
                        state["tail"] = None
                    if state["stg"] is None:
                        # reuse-Y fillers (no ACT dependency)
                        for _ in range(rpc):
                            if rq:
                                emit_y(*rq.pop(0))
                        # direct-Y, trailing one exp batch; stripe 0 must
                        # wait until all xt chunks left the shared yt slot
                        if g > 0 or k >= 7:
                            while len(dq_ready) > len(slots):
                                emit_y(*dq_ready.pop(0))
                    # normalization spread over the back stripes
                    if g == G - 2 and k == 2:
                        y_stage_box[0] = bigf32.tile(
                            [P, NB, D], F32, tag="big", name="y_stage"
                        )
                        normalize_and_store(0, 3)
                    if g == G - 1 and k == 0:
                        normalize_and_store(3, 5)
                    if g == G - 1 and k == 1:
                        normalize_and_store(5, G - 1)

                # stripe done emitting S/exp: queue leftovers as carry
                def make_carry(e, emit_y=emit_y):
                    return lambda: emit_y(*e)

                leftovers = rq + dq_ready
                state["carry"] = [make_carry(e) for e in leftovers]

                def make_stg(g=g, yt_box=yt_box, n_emitted=n_emitted):
                    def stg_fn():
                        assert n_emitted[0] == 32, (g, n_emitted[0])
                        stg = stg_pool.tile(
                            [P, 512], BF16, tag="stg", name="stg"
                        )
                        nc.vector.tensor_copy(stg[:], yt_box[0][:])
                        state["tail"] = make_tail(g, stg, fine=(g == G - 1))

                    return stg_fn

                state["stg"] = make_stg()

            # end of kernel: drain the last stripe's work
            while state["carry"]:
                state["carry"].pop(0)()
            state["stg"]()
            state["tail"]()
            normalize_and_store(G - 1, G)

    nc.compile()
    return nc


def _get_nc():
    global _CACHED_NC
    if _CACHED_NC is None:
        _CACHED_NC = _build()
    return _CACHED_NC


def kernel(X: np.ndarray, bandwidth: np.ndarray, **run_kwargs):
    """Full-input entry point: X [8, 4096, 128] f32, bandwidth scalar f32.

    Returns [8, 4096, 128] f32. Distributes one batch per NeuronCore.
    """
    X = np.ascontiguousarray(X, dtype=np.float32)
    B = X.shape[0]
    assert X.shape == (B, N, D), X.shape
    bw = np.asarray(bandwidth, dtype=np.float32).reshape(1)

    nc = _get_nc()
    in_maps = [{"X": X[b], "bandwidth": bw} for b in range(B)]
    try:
        res = run_bass_kernel_spmd(nc, in_maps, core_ids=list(range(B)), **run_kwargs)
    except Exception:
        # The first execution after other jax-on-neuron work occasionally hits
        # a transient NRT_EXEC_UNIT_UNRECOVERABLE; a retry succeeds.
        res = run_bass_kernel_spmd(nc, in_maps, core_ids=list(range(B)), **run_kwargs)
    out = np.stack([res.results[b]["Y"] for b in range(B)], axis=0)
    kernel.last_results = res
    return out


if __name__ == "__main__":
    rng = np.random.default_rng(0)
    X = rng.standard_normal((8, N, D), dtype=np.float32)
    X /= np.linalg.norm(X, axis=-1, keepdims=True)
    out = kernel(X=X, bandwidth=np.float32(0.1))
    print("out shape", out.shape, "finite", np.isfinite(out).all())


# revision 7
# speedup vs baseline: 1.3772x; 1.0539x over previous
"""Trainium2 Bass kernel for GBMS mean-shift step (nn_GBMS_RNN_137438953906).

Math (per batch b):
    W = exp((X X^T - 1) / bandwidth^2)          [N, N]
    Y = (W @ X) / rowsum(W)                     [N, D]
    out = Y / max(||Y||_2, 1e-12)  (L2 norm along D)

rowsum(W) is a positive per-row scalar, so it cancels in the final L2
normalization; we never compute row sums.  Uniform scales on X cancel the
same way, so X is carried as 8*X (fp8-friendly range, exact power of 2).

Sharding: data-parallel over batch B=8 across the 8 NeuronCores.

Per-core dataflow (N=4096 as 8 column stripes of 512; W tiles are
[128 j-rows x 512 stripe-cols], 32 j-blocks per stripe):
  xt8[d64, 2, n] = 8*X^T in fp8e4m3  (PE half-transposes of bf16 8X + DVE
      convert; the [64,2] split is the DoubleRow matmul's paired-K layout)
  direct tile (jb, g):  S = xt8_jb^T xt8_g   (fp8 DoubleRow, 0.5 cyc/row)
                        W = exp(S/(64 b^2) - 1/b^2) -> bf16
                        (ACT, 3-tile batches, runtime scale/bias APs)
  symmetry reuse: W is symmetric, so the 4g tiles of stripe g above the
      diagonal are never recomputed: when stripe g' finishes the 4-tile
      group destined for stripe gd, ONE wide XBAR DMA-transpose turns the
      group [128, 2048] into wr [128, 16, 128] whose strided views
      wr[:, q::4, :] are ready-to-use transposed rhs tiles for stripe gd.
      This removes 44% of the exp work (ACT is the co-bottleneck with PE)
      and 44% of the S matmuls, at zero PE/ACT cost (DMA+HWDGE are idle).
  Y accumulation: yt[d, n512] += x16_jb^T @ W_tile  (bf16 matmuls, PSUM).
  Tail per stripe: yt -> bf16 stage -> PE transpose -> y16[n, d]; squares
      + row-reduce on the otherwise idle Pool engine; fast-inverse-sqrt
      normalization (DVE bit trick + 2 Newton steps); f32 stores.

fp8/bf16 error budget (worst case b=1.0): fp8 X quantization perturbs the
exponent by ~4.5e-3 rms -> ~0.5% output; bf16 W and bf16 X add ~0.1% each.
At b=0.1 the diagonal dominates W and the output is bf16(x_n) exactly.
"""

import sys

if "/opt/trn_rl_repo" not in sys.path:
    sys.path.insert(0, "/opt/trn_rl_repo")

import numpy as np

import concourse.mybir as mybir
from concourse import bacc
from concourse.tile import TileContext
from concourse.bass_utils import run_bass_kernel_spmd
from concourse.masks import make_identity

P = 128
N = 4096
D = 128
NB = N // P  # 32 row blocks
G = N // 512  # 8 column stripes
NCHUNK = 8  # input DMA chunks (4 row-blocks each)

F32 = mybir.dt.float32
BF16 = mybir.dt.bfloat16
FP8 = mybir.dt.float8e4
I32 = mybir.dt.int32
DR = mybir.MatmulPerfMode.DoubleRow

_CACHED_NC = None


def _build():
    nc = bacc.Bacc("TRN2", target_bir_lowering=False, debug=False)

    x_in = nc.dram_tensor("X", [N, D], F32, kind="ExternalInput")
    bw_in = nc.dram_tensor("bandwidth", [1], F32, kind="ExternalInput")
    y_out = nc.dram_tensor("Y", [N, D], F32, kind="ExternalOutput")

    x_src = x_in.rearrange("(jb p) d -> p jb d", p=P)  # [128, 32, 128] view
    y_dst = y_out.rearrange("(nb p) d -> p nb d", p=P)

    with TileContext(nc) as tc:
        with (
            tc.tile_pool(name="const", bufs=1) as const,
            tc.tile_pool(name="bigf32", bufs=1) as bigf32,
            tc.tile_pool(name="svpool", bufs=2) as sv_pool,
            tc.tile_pool(name="wrpool", bufs=17) as wr_pool,
            tc.tile_pool(name="sqpool", bufs=2) as sq_pool,
            tc.tile_pool(name="stgpool", bufs=2) as stg_pool,
            tc.tile_pool(name="spsum", bufs=2, space="PSUM") as s_pool,
            tc.tile_pool(name="ytpsum", bufs=2, space="PSUM") as yt_pool,
            tc.tile_pool(name="tppsum", bufs=2, space="PSUM") as tp_pool,
        ):
            # ---- input DMAs: chunk 0 first (it gates the pipeline) ----
            x_nat = bigf32.tile([P, NB, D], F32, tag="big", name="x_nat")
            cb = NB // NCHUNK  # 4 row blocks per chunk
            nc.sync.dma_start(x_nat[:, 0:cb, :], x_src[:, 0:cb, :])

            bw = const.tile([P, 1], F32)
            nc.gpsimd.dma_start(bw[:], bw_in[None, :].to_broadcast([P, 1]))

            # identity built on the Pool engine (no DMA slot needed)
            ident = const.tile([P, P], F32)
            make_identity(nc, ident[:])
            identb = const.tile([P, P], BF16)
            nc.vector.tensor_copy(identb[:], ident[:])

            for c in range(1, NCHUNK):
                nc.sync.dma_start(
                    x_nat[:, c * cb : (c + 1) * cb, :],
                    x_src[:, c * cb : (c + 1) * cb, :],
                )

            # ---- runtime scalars ----
            scr = const.tile([P, 5], F32)
            bsq = scr[:, 0:1]
            rb2 = scr[:, 1:2]
            negc = scr[:, 2:3]
            sc64 = scr[:, 3:4]
            dummy = scr[:, 4:5]
            nc.vector.tensor_tensor(bsq, bw[:], bw[:], mybir.AluOpType.mult)
            nc.vector.reciprocal(rb2, bsq)  # 1/b^2
            nc.vector.tensor_scalar_mul(negc, rb2, -1.0)  # -1/b^2
            nc.vector.tensor_scalar_mul(sc64, rb2, 1.0 / 64.0)  # 1/(64 b^2)

            # preload the Exp ACT table while DMAs stream in
            nc.scalar.activation(dummy, bsq, mybir.ActivationFunctionType.Exp)

            x16 = const.tile([P, NB, D], BF16)  # 8*X, Y-matmul lhsT
            xt8 = const.tile([64, 2, N], FP8)  # 8*X^T, S-matmul operands

            # PE warm-up junk transposes (ramp the PE clock during DMA wait)
            warm = s_pool.tile([P, 2, 512], F32, tag="s", name="warm")
            for t in range(6):
                nc.tensor.transpose(
                    warm[:, t // 3, (t % 3) * P : (t % 3 + 1) * P],
                    ident[:],
                    ident[:],
                )

            chunks_done = [0]

            def emit_chunk(c):
                blk = slice(c * cb, (c + 1) * cb)
                nc.vector.tensor_scalar_mul(x16[:, blk, :], x_nat[:, blk, :], 8.0)
                xtp = tp_pool.tile([64, 2, 512], BF16, tag="tp", name="xtp")
                for o in range(cb):
                    jb = c * cb + o
                    for i in range(2):
                        nc.tensor.transpose(
                            xtp[:, i, o * P : (o + 1) * P],
                            x16[:, jb, i * 64 : (i + 1) * 64],
                            identb[:],
                        )
                nc.vector.tensor_copy(xt8[:, :, c * 512 : (c + 1) * 512], xtp[:])

            def need_chunks(upto):
                while chunks_done[0] <= min(upto, NCHUNK - 1):
                    emit_chunk(chunks_done[0])
                    chunks_done[0] += 1

            # ---- output staging ----
            y16 = const.tile([P, NB, D], BF16)  # [n_in_block, nb, d]
            ss_all = const.tile([P, NB], F32)
            half = const.tile([P, NB], F32)
            tmp = const.tile([P, NB], F32)
            rcp = const.tile([P, NB], F32)
            magic = const.tile([P, NB], I32)
            shreg = const.tile([P, NB], I32)
            nc.vector.memset(magic[:], 0x5F3759DF)
            y_stage_box = [None]

            def normalize_and_store(g0, g1):
                """L2-normalize output rows of stripes [g0, g1) and DMA out.
                1/norm = rsqrt(ss) via fast-inverse-sqrt + 2 Newton steps
                (DVE-only).  ss == 0 rows stay finite, matching the
                reference's eps-guarded division."""
                y_stage = y_stage_box[0]
                lo, hi = g0 * 4, g1 * 4  # nb range
                ss = ss_all[:, lo:hi]
                rs = rcp[:, lo:hi]
                hf = half[:, lo:hi]
                tm = tmp[:, lo:hi]
                nc.vector.tensor_scalar(
                    shreg[:, lo:hi],
                    ss.bitcast(I32),
                    1,
                    None,
                    mybir.AluOpType.logical_shift_right,
                )
                nc.vector.tensor_tensor(
                    rs.bitcast(I32),
                    magic[:, lo:hi],
                    shreg[:, lo:hi],
                    mybir.AluOpType.subtract,
                )
                nc.vector.tensor_scalar_mul(hf, ss, 0.5)
                for _ in range(2):
                    nc.vector.tensor_tensor(tm, rs, rs, mybir.AluOpType.mult)
                    nc.vector.tensor_tensor(tm, tm, hf, mybir.AluOpType.mult)
                    nc.vector.tensor_scalar(
                        tm, tm, -1.0, 1.5, mybir.AluOpType.mult, mybir.AluOpType.add
                    )
                    nc.vector.tensor_tensor(rs, rs, tm, mybir.AluOpType.mult)
                for nb in range(lo, hi):
                    nc.vector.tensor_scalar_mul(
                        y_stage[:, nb, :], y16[:, nb, :], rcp[:, nb : nb + 1]
                    )
                mid = (lo + hi) // 2
                nc.sync.dma_start(y_dst[:, lo:mid, :], y_stage[:, lo:mid, :])
                nc.sync.dma_start(y_dst[:, mid:hi, :], y_stage[:, mid:hi, :])

            def make_tail(g, stg, fine=False):
                """Tail of stripe g: stg (= yt in bf16) -> PE transpose ->
                y16[n, d]; squares on the idle Pool engine (all-DVE per-half
                pipeline in fine mode for the end-of-kernel critical path)."""

                def tail():
                    halves = 2 if fine else 1
                    hw_ = 4 // halves
                    for h in range(halves):
                        tp = tp_pool.tile([P, 4, P], BF16, tag="tp", name="tp")
                        for t in range(hw_):
                            tt = h * hw_ + t
                            nc.tensor.transpose(
                                tp[:, t, :],
                                stg[:, tt * P : (tt + 1) * P],
                                identb[:],
                            )
                        nbs = slice(g * 4 + h * hw_, g * 4 + (h + 1) * hw_)
                        nc.vector.tensor_copy(y16[:, nbs, :], tp[:, 0:hw_, :])
                        if fine:
                            sqt = sq_pool.tile([P, 4, P], F32, tag="sq", name="sqt")
                            nc.vector.tensor_tensor(
                                sqt[:, 0:hw_, :],
                                y16[:, nbs, :],
                                y16[:, nbs, :],
                                mybir.AluOpType.mult,
                            )
                            nc.vector.tensor_reduce(
                                ss_all[:, nbs],
                                sqt[:, 0:hw_, :],
                                axis=mybir.AxisListType.X,
                                op=mybir.AluOpType.add,
                            )
                    if not fine:
                        sqt = sq_pool.tile([P, 4, P], F32, tag="sq", name="sqt")
                        nbs = slice(g * 4, g * 4 + 4)
                        nc.gpsimd.tensor_tensor(
                            sqt[:], y16[:, nbs, :], y16[:, nbs, :],
                            mybir.AluOpType.mult,
                        )
                        nc.vector.tensor_reduce(
                            ss_all[:, nbs],
                            sqt[:],
                            axis=mybir.AxisListType.X,
                            op=mybir.AluOpType.add,
                        )

                return tail

            wr_tiles = {}  # (gs, gd) -> wide-transposed 4-tile group
            # cross-stripe pipeline state: leftover Y matmuls of the previous
            # stripe (drained a few per cycle so the next stripe's S/exp are
            # never stuck behind a matmul burst), then its stg copy + tail.
            state = {"carry": [], "stg": None, "tail": None}
            CR = 8  # carry drain rate per cycle

            # ---- main loop over column stripes ----
            for g in range(G):
                ndirect = 32 - 4 * g

                # direct j-blocks 4g..31 in exp batches of 2 (PSUM gives
                # the s pipeline 2x2 banks, freeing 2 banks for a double-
                # buffered yt so consecutive stripes' Y overlap)
                batches = [[s, s + 1] for s in range(0, ndirect, 2)]

                sv = sv_pool.tile([P, 32, 512], BF16, tag="sv", name="sv")
                yt_box = [None]
                n_emitted = [0]

                def emit_y(jb, rhs, yt_box=yt_box, n_emitted=n_emitted):
                    if yt_box[0] is None:
                        # lazy: the yt slot is shared with the odd xt chunks
                        # at startup and with the previous stripe's yt (whose
                        # stg copy must be emitted first)
                        yt_box[0] = yt_pool.tile(
                            [P, 512], F32, tag="yt", name="yt"
                        )
                    nc.tensor.matmul(
                        yt_box[0][:],
                        x16[:, jb, :],
                        rhs,
                        start=(n_emitted[0] == 0),
                        stop=(n_emitted[0] == 31),
                    )
                    n_emitted[0] += 1

                rq = []
                for jb in range(4 * g):
                    gs, q = jb // 4, jb % 4
                    rq.append((jb, wr_tiles[(gs, g)][:, q:16:4, :]))
                rpc = -(-len(rq) // len(batches))  # ceil: spread over cycles

                dq_ready = []  # direct (jb, rhs) whose exp has been emitted

                for k, slots in enumerate(batches):
                    if g == 0:
                        need_chunks(min((2 * k + 1) // cb + 1, NCHUNK - 1))
                    # S matmuls for batch k (fp8 DoubleRow)
                    s_t = s_pool.tile([P, 2, 512], F32, tag="s", name="s_t")
                    for q, sl in enumerate(slots):
                        jb = 4 * g + sl
                        nc.tensor.matmul(
                            s_t[:, q, :],
                            xt8[:, :, jb * P : (jb + 1) * P],
                            xt8[:, :, g * 512 : (g + 1) * 512],
                            start=True,
                            stop=True,
                            perf_mode=DR,
                        )
                    # exp batch k -> sv slots (bf16)
                    nc.scalar.activation(
                        sv[:, slots[0] : slots[-1] + 1, :],
                        s_t[:, 0 : len(slots), :],
                        mybir.ActivationFunctionType.Exp,
                        bias=negc,
                        scale=sc64,
                    )
                    for sl in slots:
                        dq_ready.append((4 * g + sl, sv[:, sl, :]))
                        # 4-tile group complete -> wide DMA transpose for
                        # the stripe it serves
                        if sl % 4 == 3 and sl >= 4:
                            gd = g + sl // 4
                            wr = wr_pool.tile(
                                [P, 16, P], BF16, tag="wr", name="wr"
                            )
                            wr_tiles[(g, gd)] = wr
                            nc.sync.dma_start_transpose(
                                wr[:],
                                sv[:, sl - 3 : sl + 1, :].rearrange(
                                    "p a b -> p (a b)"
                                ),
                            )
                    # drain the previous stripe's leftovers, then its stg
                    # copy + tail (frees the shared yt slot for this stripe)
                    for _ in range(CR):
                        if state["carry"]:
                            state["carry"].pop(0)()
                    if not state["carry"] and state["stg"] is not None:
                        state["stg"]()
                        state["stg"] = None
                        state["tail"]()
                        state["tail"] = None
                    # reuse-Y fillers (no ACT dependency)
                    for _ in range(rpc):
                        if rq:
                            emit_y(*rq.pop(0))
                    # direct-Y, trailing one exp batch
                    while len(dq_ready) > len(slots):
                        emit_y(*dq_ready.pop(0))
                    # normalization spread over the back stripes
                    if g == G - 2 and k == 3:
                        y_stage_box[0] = bigf32.tile(
                            [P, NB, D], F32, tag="big", name="y_stage"
                        )
                        normalize_and_store(0, 3)
                    if g == G - 1 and k == 0:
                        normalize_and_store(3, 5)
                    if g == G - 1 and k == 1:
                        normalize_and_store(5, G - 1)

                # stripe done emitting S/exp: queue leftovers as carry
                def make_carry(e, emit_y=emit_y):
                    return lambda: emit_y(*e)

                leftovers = rq + dq_ready
                state["carry"] = [make_carry(e) for e in leftovers]

                def make_stg(g=g, yt_box=yt_box, n_emitted=n_emitted):
                    def stg_fn():
                        assert n_emitted[0] == 32, (g, n_emitted[0])
                        stg = stg_pool.tile(
                            [P, 512], BF16, tag="stg", name="stg"
                        )
                        nc.vector.tensor_copy(stg[:], yt_box[0][:])
                        state["tail"] = make_tail(g, stg, fine=(g == G - 1))

                    return stg_fn

                state["stg"] = make_stg()

            # end of kernel: drain the last stripe's work
            while state["carry"]:
                state["carry"].pop(0)()
            state["stg"]()
            state["tail"]()
            normalize_and_store(G - 1, G)

    nc.compile()
    return nc


def _get_nc():
    global _CACHED_NC
    if _CACHED_NC is None:
        _CACHED_NC = _build()
    return _CACHED_NC


def kernel(X: np.ndarray, bandwidth: np.ndarray, **run_kwargs):
    """Full-input entry point: X [8, 4096, 128] f32, bandwidth scalar f32.

    Returns [8, 4096, 128] f32. Distributes one batch per NeuronCore.
    """
    X = np.ascontiguousarray(X, dtype=np.float32)
    B = X.shape[0]
    assert X.shape == (B, N, D), X.shape
    bw = np.asarray(bandwidth, dtype=np.float32).reshape(1)

    nc = _get_nc()
    in_maps = [{"X": X[b], "bandwidth": bw} for b in range(B)]
    try:
        res = run_bass_kernel_spmd(nc, in_maps, core_ids=list(range(B)), **run_kwargs)
    except Exception:
        # The first execution after other jax-on-neuron work occasionally hits
        # a transient NRT_EXEC_UNIT_UNRECOVERABLE; a retry succeeds.
        res = run_bass_kernel_spmd(nc, in_maps, core_ids=list(range(B)), **run_kwargs)
    out = np.stack([res.results[b]["Y"] for b in range(B)], axis=0)
    kernel.last_results = res
    return out


if __name__ == "__main__":
    rng = np.random.default_rng(0)
    X = rng.standard_normal((8, N, D), dtype=np.float32)
    X /= np.linalg.norm(X, axis=-1, keepdims=True)
    out = kernel(X=X, bandwidth=np.float32(0.1))
    print("out shape", out.shape, "finite", np.isfinite(out).all())
